# revision 28
# baseline (speedup 1.0000x reference)
"""Trainium2 Bass kernel for nn_CachedCompressedLinear.

out[16, 11008] = x[16, 4096] @ ((w_q - 128) * scale).T + bias

Sharding: column-parallel over 8 NeuronCores; each core computes a
[16, 1376] slice of the output (11008 = 8 * 1376).

vs the int32 baseline: the quantized codes fit in ONE byte, so the host
re-encodes w_q as (w_q - 128).astype(int8) -- lossless, 4x less HBM
traffic (22.5 MB -> 5.63 MB per core; total DMA bytes are the scarce
resource, the stream rate does not scale with extra load).  On device
the decode is a pure int8 -> bf16 copy (codes |v| <= 128 are exact in
bf16) split between DVE and ACT only: GpSimd stays OFF the decode path
because its SBUF port is the shared pair DVE's 2x perf mode needs
(exclusive lock, the loser fully blocks).  The dequant scale is folded
into x on the host (s commutes through the matmul), so PSUM holds
s*(x@W) and the epilogue is a single DVE add of the f32 bias per
chunk.  x is replicated in bf16 (error ~1.6e-3 rel, tolerance 2e-2).
A ladder of warmup matmuls on a memset tile keeps the PE HAM
clock-gate warm until real weights arrive.
"""

import sys

if "/opt/trn_rl_repo" not in sys.path:
    sys.path.insert(0, "/opt/trn_rl_repo")

import numpy as np
import ml_dtypes

IN_F = 4096
OUT_F = 11008
BATCH = 16
N_CORES = 8
O_PER = 1376  # out_features per core (11008 = 8 * 1376)
K_TILES = 32  # 4096 / 128
M = 16  # stationary columns: x in bf16
# chunk2 == ACT's decode region, so chunk0/1 matmuls depend only on DVE
# and chunk2 only on ACT (clean producer->consumer edges)
CHUNKS = [(0, 480), (480, 480), (960, 416)]  # PSUM-bank-sized o-chunks

DVE_W = 960  # decode split: DVE cols [0:960), ACT cols [960:1376)

# weight DMA schedule over the first 30 k-tiles: small groups first for
# fast pipeline startup, then quads; k30/k31 are DMAed chunk-wise so each
# output chunk can close as soon as its own tail slice lands.
SCHED = [(0, 1), (1, 1), (2, 2), (4, 4), (8, 4), (12, 4), (16, 4), (20, 4),
         (24, 4), (28, 2)]
K_TAIL = [30, 31]
TAIL_ENG = {0: "v", 1: "v", 2: "a"}  # chunk -> decode engine for the tail

# PE warmup ladder: big matmuls while surely idle, small ones near the
# handoff to real work (fine granularity, no oversized queue delay)
WARM_LADDER = [512] * 4 + [256] * 4 + [128] * 6

_BUILT = None


def _build():
    """Build the (SPMD, per-core) Bass program once."""
    import concourse.bass as bass
    import concourse.tile as tile
    from concourse import bacc, mybir

    dt = mybir.dt
    nc = bacc.Bacc("TRN2", target_bir_lowering=False, debug=False)

    w8 = nc.dram_tensor("w8", [128, K_TILES, O_PER], dt.int8,
                        kind="ExternalInput")
    xt = nc.dram_tensor("xt", [128, K_TILES * M], dt.bfloat16,
                        kind="ExternalInput")
    bias_rep = nc.dram_tensor("bias_rep", [BATCH, O_PER], dt.float32,
                              kind="ExternalInput")
    out = nc.dram_tensor("out", [BATCH, O_PER], dt.float32,
                         kind="ExternalOutput")

    with tile.TileContext(nc) as tc:
        with (
            tc.tile_pool(name="consts", bufs=1) as consts,
            tc.tile_pool(name="w8p", bufs=1) as w8p,
            tc.tile_pool(name="wbfp", bufs=1) as wbfp,
            tc.tile_pool(name="psum", bufs=1, space=bass.MemorySpace.PSUM) as psump,
            tc.tile_pool(name="outp", bufs=1) as outp,
        ):
            def _copy(e, dst, src):
                # pure dtype-converting copy: the dequant scale is folded
                # into x on the host
                if e == "a":
                    nc.scalar.activation(
                        dst, src, mybir.ActivationFunctionType.Copy)
                else:
                    nc.vector.tensor_copy(dst, src)

            x_sb = consts.tile([128, K_TILES * M], dt.bfloat16)
            bias_sb = consts.tile([BATCH, O_PER], dt.float32)

            # ---- k0 rides the SWDGE path: its descriptor generation runs
            # in parallel with the SP/HWDGE stream, so the first decode
            # starts ~1us earlier
            w8_t = {}
            t0_ = w8p.tile([128, 1, O_PER], dt.int8, tag="w8_0")
            nc.gpsimd.dma_start(t0_[:], w8[:][:, 0:1, :])
            w8_t[0] = t0_
            nc.gpsimd.dma_start(bias_sb[:], bias_rep[:])

            # ---- weight stream on SP/HWDGE; x ahead of k1
            nc.sync.dma_start(x_sb[:], xt[:])
            for k0, nk in SCHED[1:]:
                t = w8p.tile([128, nk, O_PER], dt.int8, tag=f"w8_{k0}")
                nc.sync.dma_start(t[:], w8[:][:, k0:k0 + nk, :])
                w8_t[k0] = t
            # tail: k30/k31 chunk-wise
            w8_tail = {}
            for i, (o, w) in enumerate(CHUNKS):
                for k in K_TAIL:
                    t = w8p.tile([128, w], dt.int8, tag=f"w8t_{i}_{k}")
                    nc.sync.dma_start(t[:], w8[:][:, k, o:o + w])
                    w8_tail[(i, k)] = t

            # ---- PE warmup: matmuls on a zeroed tile; tiny memset so the
            # dependency resolves as early as possible
            warm_mv = consts.tile([128, 512], dt.bfloat16)
            nc.vector.memset(warm_mv[:], 0.0)
            warm_ps = psump.tile([16, 512], dt.float32, tag="warm")
            for wcols in WARM_LADDER:
                nc.tensor.matmul(warm_ps[:, 0:wcols], warm_mv[:, 0:16],
                                 warm_mv[:, 0:wcols], start=True, stop=True)

            # ---- decode int8 -> bf16 (pure dtype-converting copy)
            wbf_t = {}
            for k0, nk in SCHED:
                t = wbfp.tile([128, nk, O_PER], dt.bfloat16, tag=f"wbf_{k0}")
                wbf_t[k0] = t
            last_k0 = SCHED[-1][0]
            for k0, nk in SCHED:
                # DVE per k-pair mid-stream (amortize instruction overhead),
                # per single k in the final group for prompt release
                vstep = 1 if (nk == 1 or k0 == last_k0) else 2
                for j in range(0, nk, vstep):
                    js = slice(j, j + vstep)
                    _copy("v", wbf_t[k0][:, js, 0:DVE_W],
                          w8_t[k0][:, js, 0:DVE_W])
                # ACT per whole group: its instrs only gate chunk2
                # matmuls now (last of the three per k-tile), so coarse
                # granularity amortizes its high per-instruction init
                astep = nk if nk >= 2 else 1
                for j in range(0, nk, astep):
                    js = slice(j, j + astep)
                    _copy("a", wbf_t[k0][:, js, DVE_W:O_PER],
                          w8_t[k0][:, js, DVE_W:O_PER])
            wbf_tail = {}
            for i, (o, w) in enumerate(CHUNKS):
                for k in K_TAIL:
                    t = wbfp.tile([128, w], dt.bfloat16, tag=f"wbft_{i}_{k}")
                    _copy(TAIL_ENG[i], t[:], w8_tail[(i, k)][:])
                    wbf_tail[(i, k)] = t

            # ---- matmuls
            psums = [
                psump.tile([16, w], dt.float32, name=f"ps{i}", tag=f"ps{i}")
                for i, (_, w) in enumerate(CHUNKS)
            ]
            for k0, nk in SCHED:
                for j in range(nk):
                    k = k0 + j
                    for i, (o, w) in enumerate(CHUNKS):
                        nc.tensor.matmul(
                            psums[i][:],
                            x_sb[:, k * M:(k + 1) * M],
                            wbf_t[k0][:, j, o:o + w],
                            start=(k == 0),
                            stop=False,
                        )
            # tail, chunk-major so each chunk closes in turn
            for i, (o, w) in enumerate(CHUNKS):
                for k in K_TAIL:
                    nc.tensor.matmul(
                        psums[i][:],
                        x_sb[:, k * M:(k + 1) * M],
                        wbf_tail[(i, k)][:],
                        start=False,
                        stop=(k == K_TAIL[-1]),
                    )

            # ---- epilogue per chunk: PSUM already holds s*(x@W), so one
            # DVE add of the f32 bias closes the chunk; SP DMAs it out.
            for i, (o, w) in enumerate(CHUNKS):
                comb = outp.tile([BATCH, w], dt.float32, tag=f"comb_{i}")
                nc.vector.tensor_add(comb[:], psums[i][:], bias_sb[:, o:o + w])
                nc.sync.dma_start(out[:][:, o:o + w], comb[:])

    nc.compile()
    return nc


def _get_built():
    global _BUILT
    if _BUILT is None:
        _BUILT = _build()
    return _BUILT


def make_in_maps(x, w_q, scale, bias):
    """Host-side shard + layout prep. Returns per-core input dicts."""
    x = np.asarray(x, dtype=np.float32)
    w_q = np.asarray(w_q, dtype=np.int32)
    scale = np.asarray(scale, dtype=np.float32)
    bias = np.asarray(bias, dtype=np.float32)

    # x -> bf16 with the dequant scale folded in, packed so partition p
    # holds, for each k-tile t, the stationary row (t*128 + p): [128, 32*16]
    s_val = scale.reshape(-1)[0]
    xT = np.ascontiguousarray(x.T * s_val).astype(ml_dtypes.bfloat16)
    xt = np.ascontiguousarray(
        xT.reshape(K_TILES, 128, M).transpose(1, 0, 2)
    ).reshape(128, K_TILES * M)

    # codes -> int8 (lossless: w_q in [0,255], shift to [-128,127])
    w8_full = (w_q - 128).astype(np.int8)  # [11008, 4096]

    in_maps = []
    for c in range(N_CORES):
        sl = w8_full[c * O_PER:(c + 1) * O_PER]  # [1376, 4096]
        # [128, 32, 1376]: partition p, (k, f) = W[f, k*128 + p]
        w8c = np.ascontiguousarray(
            sl.T.reshape(K_TILES, 128, O_PER).transpose(1, 0, 2)
        )
        bias_c = np.ascontiguousarray(
            np.broadcast_to(bias[c * O_PER:(c + 1) * O_PER], (BATCH, O_PER))
        )
        in_maps.append(
            {"w8": w8c, "xt": xt, "bias_rep": bias_c}
        )
    return in_maps


def run(inputs, trace=False):
    """Run on the 8 NeuronCores. Returns (full_output, BassKernelResults)."""
    from concourse.bass_utils import run_bass_kernel_spmd

    in_maps = make_in_maps(**inputs)
    nc = _get_built()
    res = run_bass_kernel_spmd(nc, in_maps, list(range(N_CORES)), trace=trace)
    parts = [np.asarray(res.results[c]["out"]) for c in range(N_CORES)]
    full = np.concatenate(parts, axis=1)[:, :OUT_F].astype(np.float32)
    return full, res


def kernel(**inputs) -> np.ndarray:
    full, _ = run(inputs, trace=False)
    return full


# revision 31
# speedup vs baseline: 1.0737x; 1.0737x over previous
"""Trainium2 Bass kernel for nn_CachedCompressedLinear.

out[16, 11008] = x[16, 4096] @ ((w_q - 128) * scale).T + bias

Sharding: column-parallel over 8 NeuronCores; each core computes a
[16, 1376] slice of the output (11008 = 8 * 1376).

vs the int32 baseline: the quantized codes fit in ONE byte, so the host
re-encodes w_q as (w_q - 128).astype(int8) -- lossless, 4x less HBM
traffic (22.5 MB -> 5.63 MB per core; total DMA bytes are the scarce
resource, the stream rate does not scale with extra load).  On device
the decode is a pure int8 -> bf16 copy (codes |v| <= 128 are exact in
bf16) split between DVE and ACT only: GpSimd stays OFF the decode path
because its SBUF port is the shared pair DVE's 2x perf mode needs
(exclusive lock, the loser fully blocks).  The dequant scale is folded
into x on the host (s commutes through the matmul), so PSUM holds
s*(x@W) and the epilogue is a single DVE add of the f32 bias per
chunk.  x is replicated in bf16 (error ~1.6e-3 rel, tolerance 2e-2).
A ladder of warmup matmuls on a memset tile keeps the PE HAM
clock-gate warm until real weights arrive.
"""

import sys

if "/opt/trn_rl_repo" not in sys.path:
    sys.path.insert(0, "/opt/trn_rl_repo")

import numpy as np
import ml_dtypes

IN_F = 4096
OUT_F = 11008
BATCH = 16
N_CORES = 8
O_PER = 1376  # out_features per core (11008 = 8 * 1376)
K_TILES = 32  # 4096 / 128
M = 16  # stationary columns: x in bf16
CHUNKS = [(0, 512), (512, 512), (1024, 352)]  # PSUM-bank-sized o-chunks

DVE_W = 960  # decode split: DVE cols [0:960), ACT cols [960:1376)

# weight DMA schedule over the first 30 k-tiles: small groups first for
# fast pipeline startup, then quads; k30/k31 are DMAed chunk-wise so each
# output chunk can close as soon as its own tail slice lands.
SCHED = [(0, 1), (1, 1), (2, 2), (4, 4), (8, 4), (12, 4), (16, 4), (20, 4),
         (24, 4), (28, 2)]
K_TAIL = [30, 31]
TAIL_ENG = {0: "v", 1: "a", 2: "v"}  # chunk -> decode engine for the tail

# PE warmup ladder: big matmuls while surely idle, small ones near the
# handoff to real work (fine granularity, no oversized queue delay)
WARM_LADDER = [512] * 4 + [256] * 4 + [128] * 6

_BUILT = None


def _build():
    """Build the (SPMD, per-core) Bass program once."""
    import concourse.bass as bass
    import concourse.tile as tile
    from concourse import bacc, mybir

    dt = mybir.dt
    nc = bacc.Bacc("TRN2", target_bir_lowering=False, debug=False)

    w8 = nc.dram_tensor("w8", [128, K_TILES, O_PER], dt.int8,
                        kind="ExternalInput")
    xt = nc.dram_tensor("xt", [128, K_TILES * M], dt.bfloat16,
                        kind="ExternalInput")
    bias_rep = nc.dram_tensor("bias_rep", [BATCH, O_PER], dt.float32,
                              kind="ExternalInput")
    out = nc.dram_tensor("out", [BATCH, O_PER], dt.float32,
                         kind="ExternalOutput")

    with tile.TileContext(nc) as tc:
        with (
            tc.tile_pool(name="consts", bufs=1) as consts,
            tc.tile_pool(name="w8p", bufs=1) as w8p,
            tc.tile_pool(name="wbfp", bufs=1) as wbfp,
            tc.tile_pool(name="psum", bufs=1, space=bass.MemorySpace.PSUM) as psump,
            tc.tile_pool(name="outp", bufs=1) as outp,
        ):
            def _copy(e, dst, src):
                # pure dtype-converting copy: the dequant scale is folded
                # into x on the host
                if e == "a":
                    nc.scalar.activation(
                        dst, src, mybir.ActivationFunctionType.Copy)
                else:
                    nc.vector.tensor_copy(dst, src)

            x_sb = consts.tile([128, K_TILES * M], dt.bfloat16)
            bias_sb = consts.tile([BATCH, O_PER], dt.float32)

            # ---- k0 rides the SWDGE path: its descriptor generation runs
            # in parallel with the SP/HWDGE stream, so the first decode
            # starts ~1us earlier
            w8_t = {}
            t0_ = w8p.tile([128, 1, O_PER], dt.int8, tag="w8_0")
            nc.gpsimd.dma_start(t0_[:], w8[:][:, 0:1, :])
            w8_t[0] = t0_
            nc.gpsimd.dma_start(bias_sb[:], bias_rep[:])

            # ---- weight stream on SP/HWDGE; x ahead of k1
            nc.sync.dma_start(x_sb[:], xt[:])
            for k0, nk in SCHED[1:]:
                t = w8p.tile([128, nk, O_PER], dt.int8, tag=f"w8_{k0}")
                nc.sync.dma_start(t[:], w8[:][:, k0:k0 + nk, :])
                w8_t[k0] = t
            # tail: k30/k31 chunk-wise
            w8_tail = {}
            for i, (o, w) in enumerate(CHUNKS):
                for k in K_TAIL:
                    t = w8p.tile([128, w], dt.int8, tag=f"w8t_{i}_{k}")
                    nc.sync.dma_start(t[:], w8[:][:, k, o:o + w])
                    w8_tail[(i, k)] = t

            # ---- PE warmup: matmuls on a zeroed tile; tiny memset so the
            # dependency resolves as early as possible
            warm_mv = consts.tile([128, 512], dt.bfloat16)
            nc.vector.memset(warm_mv[:], 0.0)
            warm_ps = psump.tile([16, 512], dt.float32, tag="warm")
            for wcols in WARM_LADDER:
                nc.tensor.matmul(warm_ps[:, 0:wcols], warm_mv[:, 0:16],
                                 warm_mv[:, 0:wcols], start=True, stop=True)

            # ---- decode int8 -> bf16 (pure dtype-converting copy)
            wbf_t = {}
            for k0, nk in SCHED:
                t = wbfp.tile([128, nk, O_PER], dt.bfloat16, tag=f"wbf_{k0}")
                wbf_t[k0] = t
            last_k0 = SCHED[-1][0]
            for k0, nk in SCHED:
                # DVE per k-pair mid-stream (amortize instruction overhead),
                # per single k in the final group for prompt release
                vstep = 1 if (nk == 1 or k0 == last_k0) else 2
                for j in range(0, nk, vstep):
                    js = slice(j, j + vstep)
                    _copy("v", wbf_t[k0][:, js, 0:DVE_W],
                          w8_t[k0][:, js, 0:DVE_W])
                # ACT per k-pair (its per-instruction overhead is high,
                # but whole-group instrs stall the chunk1/2 matmuls)
                astep = 2 if nk >= 2 else 1
                for j in range(0, nk, astep):
                    js = slice(j, j + astep)
                    _copy("a", wbf_t[k0][:, js, DVE_W:O_PER],
                          w8_t[k0][:, js, DVE_W:O_PER])
            wbf_tail = {}
            for i, (o, w) in enumerate(CHUNKS):
                for k in K_TAIL:
                    t = wbfp.tile([128, w], dt.bfloat16, tag=f"wbft_{i}_{k}")
                    _copy(TAIL_ENG[i], t[:], w8_tail[(i, k)][:])
                    wbf_tail[(i, k)] = t

            # ---- matmuls
            psums = [
                psump.tile([16, w], dt.float32, name=f"ps{i}", tag=f"ps{i}")
                for i, (_, w) in enumerate(CHUNKS)
            ]
            for k0, nk in SCHED:
                for j in range(nk):
                    k = k0 + j
                    for i, (o, w) in enumerate(CHUNKS):
                        nc.tensor.matmul(
                            psums[i][:],
                            x_sb[:, k * M:(k + 1) * M],
                            wbf_t[k0][:, j, o:o + w],
                            start=(k == 0),
                            stop=False,
                        )
            # tail, chunk-major so each chunk closes in turn
            for i, (o, w) in enumerate(CHUNKS):
                for k in K_TAIL:
                    nc.tensor.matmul(
                        psums[i][:],
                        x_sb[:, k * M:(k + 1) * M],
                        wbf_tail[(i, k)][:],
                        start=False,
                        stop=(k == K_TAIL[-1]),
                    )

            # ---- epilogue per chunk: PSUM already holds s*(x@W), so one
            # DVE add of the f32 bias closes the chunk; SP DMAs it out.
            for i, (o, w) in enumerate(CHUNKS):
                comb = outp.tile([BATCH, w], dt.float32, tag=f"comb_{i}")
                nc.vector.tensor_add(comb[:], psums[i][:], bias_sb[:, o:o + w])
                nc.sync.dma_start(out[:][:, o:o + w], comb[:])

    nc.compile()
    return nc


def _get_built():
    global _BUILT
    if _BUILT is None:
        _BUILT = _build()
    return _BUILT


def make_in_maps(x, w_q, scale, bias):
    """Host-side shard + layout prep. Returns per-core input dicts."""
    x = np.asarray(x, dtype=np.float32)
    w_q = np.asarray(w_q, dtype=np.int32)
    scale = np.asarray(scale, dtype=np.float32)
    bias = np.asarray(bias, dtype=np.float32)

    # x -> bf16 with the dequant scale folded in, packed so partition p
    # holds, for each k-tile t, the stationary row (t*128 + p): [128, 32*16]
    s_val = scale.reshape(-1)[0]
    xT = np.ascontiguousarray(x.T * s_val).astype(ml_dtypes.bfloat16)
    xt = np.ascontiguousarray(
        xT.reshape(K_TILES, 128, M).transpose(1, 0, 2)
    ).reshape(128, K_TILES * M)

    # codes -> int8 (lossless: w_q in [0,255], shift to [-128,127])
    w8_full = (w_q - 128).astype(np.int8)  # [11008, 4096]

    in_maps = []
    for c in range(N_CORES):
        sl = w8_full[c * O_PER:(c + 1) * O_PER]  # [1376, 4096]
        # [128, 32, 1376]: partition p, (k, f) = W[f, k*128 + p]
        w8c = np.ascontiguousarray(
            sl.T.reshape(K_TILES, 128, O_PER).transpose(1, 0, 2)
        )
        bias_c = np.ascontiguousarray(
            np.broadcast_to(bias[c * O_PER:(c + 1) * O_PER], (BATCH, O_PER))
        )
        in_maps.append(
            {"w8": w8c, "xt": xt, "bias_rep": bias_c}
        )
    return in_maps


def run(inputs, trace=False):
    """Run on the 8 NeuronCores. Returns (full_output, BassKernelResults)."""
    from concourse.bass_utils import run_bass_kernel_spmd

    in_maps = make_in_maps(**inputs)
    nc = _get_built()
    res = run_bass_kernel_spmd(nc, in_maps, list(range(N_CORES)), trace=trace)
    parts = [np.asarray(res.results[c]["out"]) for c in range(N_CORES)]
    full = np.concatenate(parts, axis=1)[:, :OUT_F].astype(np.float32)
    return full, res


def kernel(**inputs) -> np.ndarray:
    full, _ = run(inputs, trace=False)
    return full


# revision 32
# speedup vs baseline: 1.1430x; 1.0646x over previous
"""Trainium2 Bass kernel for nn_CachedCompressedLinear.

out[16, 11008] = x[16, 4096] @ ((w_q - 128) * scale).T + bias

Sharding: column-parallel over 8 NeuronCores; each core computes a
[16, 1376] slice of the output (11008 = 8 * 1376).

vs the int32 baseline: the quantized codes fit in ONE byte, so the host
re-encodes w_q as (w_q - 128).astype(int8) -- lossless, 4x less HBM
traffic (22.5 MB -> 5.63 MB per core; total DMA bytes are the scarce
resource, the stream rate does not scale with extra load).  On device
the decode is a pure int8 -> bf16 copy (codes |v| <= 128 are exact in
bf16) split between DVE and ACT only: GpSimd stays OFF the decode path
because its SBUF port is the shared pair DVE's 2x perf mode needs
(exclusive lock, the loser fully blocks).  The dequant scale is folded
into x on the host (s commutes through the matmul), so PSUM holds
s*(x@W) and the epilogue is a single DVE add of the f32 bias per
chunk.  x is replicated in bf16 (error ~1.6e-3 rel, tolerance 2e-2).
A ladder of warmup matmuls on a memset tile keeps the PE HAM
clock-gate warm until real weights arrive.
"""

import sys

if "/opt/trn_rl_repo" not in sys.path:
    sys.path.insert(0, "/opt/trn_rl_repo")

import numpy as np
import ml_dtypes

IN_F = 4096
OUT_F = 11008
BATCH = 16
N_CORES = 8
O_PER = 1376  # out_features per core (11008 = 8 * 1376)
K_TILES = 32  # 4096 / 128
M = 16  # stationary columns: x in bf16
CHUNKS = [(0, 512), (512, 512), (1024, 352)]  # PSUM-bank-sized o-chunks

DVE_W = 960  # decode split: DVE cols [0:960), ACT cols [960:1376)

# weight DMA schedule over the first 30 k-tiles: small groups first for
# fast pipeline startup, then quads; k30/k31 are DMAed chunk-wise so each
# output chunk can close as soon as its own tail slice lands.
SCHED = [(0, 1), (1, 1), (2, 2), (4, 2), (6, 2), (8, 2), (10, 2), (12, 4),
         (16, 4), (20, 4), (24, 4), (28, 2)]
K_TAIL = [30, 31]
TAIL_ENG = {0: "v", 1: "a", 2: "v"}  # chunk -> decode engine for the tail

# PE warmup ladder: big matmuls while surely idle, small ones near the
# handoff to real work (fine granularity, no oversized queue delay)
WARM_LADDER = [512] * 4 + [256] * 4 + [128] * 6

_BUILT = None


def _build():
    """Build the (SPMD, per-core) Bass program once."""
    import concourse.bass as bass
    import concourse.tile as tile
    from concourse import bacc, mybir

    dt = mybir.dt
    nc = bacc.Bacc("TRN2", target_bir_lowering=False, debug=False)

    w8 = nc.dram_tensor("w8", [128, K_TILES, O_PER], dt.int8,
                        kind="ExternalInput")
    xt = nc.dram_tensor("xt", [128, K_TILES * M], dt.bfloat16,
                        kind="ExternalInput")
    bias_rep = nc.dram_tensor("bias_rep", [BATCH, O_PER], dt.float32,
                              kind="ExternalInput")
    out = nc.dram_tensor("out", [BATCH, O_PER], dt.float32,
                         kind="ExternalOutput")

    with tile.TileContext(nc) as tc:
        with (
            tc.tile_pool(name="consts", bufs=1) as consts,
            tc.tile_pool(name="w8p", bufs=1) as w8p,
            tc.tile_pool(name="wbfp", bufs=1) as wbfp,
            tc.tile_pool(name="psum", bufs=1, space=bass.MemorySpace.PSUM) as psump,
            tc.tile_pool(name="outp", bufs=1) as outp,
        ):
            def _copy(e, dst, src):
                # pure dtype-converting copy: the dequant scale is folded
                # into x on the host
                if e == "a":
                    nc.scalar.activation(
                        dst, src, mybir.ActivationFunctionType.Copy)
                else:
                    nc.vector.tensor_copy(dst, src)

            x_sb = consts.tile([128, K_TILES * M], dt.bfloat16)
            bias_sb = consts.tile([BATCH, O_PER], dt.float32)

            # ---- k0 rides the SWDGE path: its descriptor generation runs
            # in parallel with the SP/HWDGE stream, so the first decode
            # starts ~1us earlier
            w8_t = {}
            t0_ = w8p.tile([128, 1, O_PER], dt.int8, tag="w8_0")
            nc.gpsimd.dma_start(t0_[:], w8[:][:, 0:1, :])
            w8_t[0] = t0_
            nc.gpsimd.dma_start(bias_sb[:], bias_rep[:])

            # ---- weight stream on SP/HWDGE; x ahead of k1
            nc.sync.dma_start(x_sb[:], xt[:])
            for k0, nk in SCHED[1:]:
                t = w8p.tile([128, nk, O_PER], dt.int8, tag=f"w8_{k0}")
                nc.sync.dma_start(t[:], w8[:][:, k0:k0 + nk, :])
                w8_t[k0] = t
            # tail: k30/k31 chunk-wise
            w8_tail = {}
            for i, (o, w) in enumerate(CHUNKS):
                for k in K_TAIL:
                    t = w8p.tile([128, w], dt.int8, tag=f"w8t_{i}_{k}")
                    nc.sync.dma_start(t[:], w8[:][:, k, o:o + w])
                    w8_tail[(i, k)] = t

            # ---- PE warmup: matmuls on a zeroed tile; tiny memset so the
            # dependency resolves as early as possible
            warm_mv = consts.tile([128, 512], dt.bfloat16)
            nc.vector.memset(warm_mv[:], 0.0)
            warm_ps = psump.tile([16, 512], dt.float32, tag="warm")
            for wcols in WARM_LADDER:
                nc.tensor.matmul(warm_ps[:, 0:wcols], warm_mv[:, 0:16],
                                 warm_mv[:, 0:wcols], start=True, stop=True)

            # ---- decode int8 -> bf16 (pure dtype-converting copy)
            wbf_t = {}
            for k0, nk in SCHED:
                t = wbfp.tile([128, nk, O_PER], dt.bfloat16, tag=f"wbf_{k0}")
                wbf_t[k0] = t
            last_k0 = SCHED[-1][0]
            for k0, nk in SCHED:
                # DVE per k-pair mid-stream (amortize instruction overhead),
                # per single k in the final group for prompt release
                vstep = 1 if (nk == 1 or k0 == last_k0) else 2
                for j in range(0, nk, vstep):
                    js = slice(j, j + vstep)
                    _copy("v", wbf_t[k0][:, js, 0:DVE_W],
                          w8_t[k0][:, js, 0:DVE_W])
                # ACT per k-pair (its per-instruction overhead is high,
                # but whole-group instrs stall the chunk1/2 matmuls)
                astep = 2 if nk >= 2 else 1
                for j in range(0, nk, astep):
                    js = slice(j, j + astep)
                    _copy("a", wbf_t[k0][:, js, DVE_W:O_PER],
                          w8_t[k0][:, js, DVE_W:O_PER])
            wbf_tail = {}
            for i, (o, w) in enumerate(CHUNKS):
                for k in K_TAIL:
                    t = wbfp.tile([128, w], dt.bfloat16, tag=f"wbft_{i}_{k}")
                    _copy(TAIL_ENG[i], t[:], w8_tail[(i, k)][:])
                    wbf_tail[(i, k)] = t

            # ---- matmuls
            psums = [
                psump.tile([16, w], dt.float32, name=f"ps{i}", tag=f"ps{i}")
                for i, (_, w) in enumerate(CHUNKS)
            ]
            for k0, nk in SCHED:
                for j in range(nk):
                    k = k0 + j
                    for i, (o, w) in enumerate(CHUNKS):
                        nc.tensor.matmul(
                            psums[i][:],
                            x_sb[:, k * M:(k + 1) * M],
                            wbf_t[k0][:, j, o:o + w],
                            start=(k == 0),
                            stop=False,
                        )
            # tail, chunk-major so each chunk closes in turn
            for i, (o, w) in enumerate(CHUNKS):
                for k in K_TAIL:
                    nc.tensor.matmul(
                        psums[i][:],
                        x_sb[:, k * M:(k + 1) * M],
                        wbf_tail[(i, k)][:],
                        start=False,
                        stop=(k == K_TAIL[-1]),
                    )

            # ---- epilogue per chunk: PSUM already holds s*(x@W), so one
            # DVE add of the f32 bias closes the chunk; SP DMAs it out.
            for i, (o, w) in enumerate(CHUNKS):
                comb = outp.tile([BATCH, w], dt.float32, tag=f"comb_{i}")
                nc.vector.tensor_add(comb[:], psums[i][:], bias_sb[:, o:o + w])
                nc.sync.dma_start(out[:][:, o:o + w], comb[:])

    nc.compile()
    return nc


def _get_built():
    global _BUILT
    if _BUILT is None:
        _BUILT = _build()
    return _BUILT


def make_in_maps(x, w_q, scale, bias):
    """Host-side shard + layout prep. Returns per-core input dicts."""
    x = np.asarray(x, dtype=np.float32)
    w_q = np.asarray(w_q, dtype=np.int32)
    scale = np.asarray(scale, dtype=np.float32)
    bias = np.asarray(bias, dtype=np.float32)

    # x -> bf16 with the dequant scale folded in, packed so partition p
    # holds, for each k-tile t, the stationary row (t*128 + p): [128, 32*16]
    s_val = scale.reshape(-1)[0]
    xT = np.ascontiguousarray(x.T * s_val).astype(ml_dtypes.bfloat16)
    xt = np.ascontiguousarray(
        xT.reshape(K_TILES, 128, M).transpose(1, 0, 2)
    ).reshape(128, K_TILES * M)

    # codes -> int8 (lossless: w_q in [0,255], shift to [-128,127])
    w8_full = (w_q - 128).astype(np.int8)  # [11008, 4096]

    in_maps = []
    for c in range(N_CORES):
        sl = w8_full[c * O_PER:(c + 1) * O_PER]  # [1376, 4096]
        # [128, 32, 1376]: partition p, (k, f) = W[f, k*128 + p]
        w8c = np.ascontiguousarray(
            sl.T.reshape(K_TILES, 128, O_PER).transpose(1, 0, 2)
        )
        bias_c = np.ascontiguousarray(
            np.broadcast_to(bias[c * O_PER:(c + 1) * O_PER], (BATCH, O_PER))
        )
        in_maps.append(
            {"w8": w8c, "xt": xt, "bias_rep": bias_c}
        )
    return in_maps


def run(inputs, trace=False):
    """Run on the 8 NeuronCores. Returns (full_output, BassKernelResults)."""
    from concourse.bass_utils import run_bass_kernel_spmd

    in_maps = make_in_maps(**inputs)
    nc = _get_built()
    res = run_bass_kernel_spmd(nc, in_maps, list(range(N_CORES)), trace=trace)
    parts = [np.asarray(res.results[c]["out"]) for c in range(N_CORES)]
    full = np.concatenate(parts, axis=1)[:, :OUT_F].astype(np.float32)
    return full, res


def kernel(**inputs) -> np.ndarray:
    full, _ = run(inputs, trace=False)
    return full


# revision 33
# speedup vs baseline: 1.1490x; 1.0052x over previous
"""Trainium2 Bass kernel for nn_CachedCompressedLinear.

out[16, 11008] = x[16, 4096] @ ((w_q - 128) * scale).T + bias

Sharding: column-parallel over 8 NeuronCores; each core computes a
[16, 1376] slice of the output (11008 = 8 * 1376).

vs the int32 baseline: the quantized codes fit in ONE byte, so the host
re-encodes w_q as (w_q - 128).astype(int8) -- lossless, 4x less HBM
traffic (22.5 MB -> 5.63 MB per core; total DMA bytes are the scarce
resource, the stream rate does not scale with extra load).  On device
the decode is a pure int8 -> bf16 copy (codes |v| <= 128 are exact in
bf16) split between DVE and ACT only: GpSimd stays OFF the decode path
because its SBUF port is the shared pair DVE's 2x perf mode needs
(exclusive lock, the loser fully blocks).  The dequant scale is folded
into x on the host (s commutes through the matmul), so PSUM holds
s*(x@W) and the epilogue is a single DVE add of the f32 bias per
chunk.  x is replicated in bf16 (error ~1.6e-3 rel, tolerance 2e-2).
A ladder of warmup matmuls on a memset tile keeps the PE HAM
clock-gate warm until real weights arrive.
"""

import sys

if "/opt/trn_rl_repo" not in sys.path:
    sys.path.insert(0, "/opt/trn_rl_repo")

import numpy as np
import ml_dtypes

IN_F = 4096
OUT_F = 11008
BATCH = 16
N_CORES = 8
O_PER = 1376  # out_features per core (11008 = 8 * 1376)
K_TILES = 32  # 4096 / 128
M = 16  # stationary columns: x in bf16
CHUNKS = [(0, 512), (512, 512), (1024, 352)]  # PSUM-bank-sized o-chunks

DVE_W = 960  # decode split: DVE cols [0:960), ACT cols [960:1376)

# weight DMA schedule over the first 30 k-tiles: small groups first for
# fast pipeline startup, then quads; k30/k31 are DMAed chunk-wise so each
# output chunk can close as soon as its own tail slice lands.
SCHED = [(0, 1), (1, 1), (2, 2), (4, 2), (6, 2), (8, 2), (10, 2), (12, 2),
         (14, 2), (16, 4), (20, 4), (24, 4), (28, 2)]
K_TAIL = [30, 31]
TAIL_ENG = {0: "v", 1: "a", 2: "v"}  # chunk -> decode engine for the tail

# PE warmup ladder: big matmuls while surely idle, small ones near the
# handoff to real work (fine granularity, no oversized queue delay)
WARM_LADDER = [512] * 4 + [256] * 4 + [128] * 6 + [160] * 3

_BUILT = None


def _build():
    """Build the (SPMD, per-core) Bass program once."""
    import concourse.bass as bass
    import concourse.tile as tile
    from concourse import bacc, mybir

    dt = mybir.dt
    nc = bacc.Bacc("TRN2", target_bir_lowering=False, debug=False)

    w8 = nc.dram_tensor("w8", [128, K_TILES, O_PER], dt.int8,
                        kind="ExternalInput")
    xt = nc.dram_tensor("xt", [128, K_TILES * M], dt.bfloat16,
                        kind="ExternalInput")
    bias_rep = nc.dram_tensor("bias_rep", [BATCH, O_PER], dt.float32,
                              kind="ExternalInput")
    out = nc.dram_tensor("out", [BATCH, O_PER], dt.float32,
                         kind="ExternalOutput")

    with tile.TileContext(nc) as tc:
        with (
            tc.tile_pool(name="consts", bufs=1) as consts,
            tc.tile_pool(name="w8p", bufs=1) as w8p,
            tc.tile_pool(name="wbfp", bufs=1) as wbfp,
            tc.tile_pool(name="psum", bufs=1, space=bass.MemorySpace.PSUM) as psump,
            tc.tile_pool(name="outp", bufs=1) as outp,
        ):
            def _copy(e, dst, src):
                # pure dtype-converting copy: the dequant scale is folded
                # into x on the host
                if e == "a":
                    nc.scalar.activation(
                        dst, src, mybir.ActivationFunctionType.Copy)
                else:
                    nc.vector.tensor_copy(dst, src)

            x_sb = consts.tile([128, K_TILES * M], dt.bfloat16)
            bias_sb = consts.tile([BATCH, O_PER], dt.float32)

            # ---- k0 rides the SWDGE path: its descriptor generation runs
            # in parallel with the SP/HWDGE stream, so the first decode
            # starts ~1us earlier
            w8_t = {}
            t0_ = w8p.tile([128, 1, O_PER], dt.int8, tag="w8_0")
            nc.gpsimd.dma_start(t0_[:], w8[:][:, 0:1, :])
            w8_t[0] = t0_
            nc.gpsimd.dma_start(bias_sb[:], bias_rep[:])

            # ---- weight stream on SP/HWDGE; x ahead of k1
            nc.sync.dma_start(x_sb[:], xt[:])
            for k0, nk in SCHED[1:]:
                t = w8p.tile([128, nk, O_PER], dt.int8, tag=f"w8_{k0}")
                nc.sync.dma_start(t[:], w8[:][:, k0:k0 + nk, :])
                w8_t[k0] = t
            # tail: k30/k31 chunk-wise
            w8_tail = {}
            for i, (o, w) in enumerate(CHUNKS):
                for k in K_TAIL:
                    t = w8p.tile([128, w], dt.int8, tag=f"w8t_{i}_{k}")
                    nc.sync.dma_start(t[:], w8[:][:, k, o:o + w])
                    w8_tail[(i, k)] = t

            # ---- PE warmup: matmuls on a zeroed tile; tiny memset so the
            # dependency resolves as early as possible
            warm_mv = consts.tile([128, 512], dt.bfloat16)
            nc.vector.memset(warm_mv[:], 0.0)
            warm_ps = psump.tile([16, 512], dt.float32, tag="warm")
            for wcols in WARM_LADDER:
                nc.tensor.matmul(warm_ps[:, 0:wcols], warm_mv[:, 0:16],
                                 warm_mv[:, 0:wcols], start=True, stop=True)

            # ---- decode int8 -> bf16 (pure dtype-converting copy)
            wbf_t = {}
            for k0, nk in SCHED:
                t = wbfp.tile([128, nk, O_PER], dt.bfloat16, tag=f"wbf_{k0}")
                wbf_t[k0] = t
            last_k0 = SCHED[-1][0]
            for k0, nk in SCHED:
                # DVE per k-pair mid-stream (amortize instruction overhead),
                # per single k in the final group for prompt release
                vstep = 1 if (nk == 1 or k0 == last_k0) else 2
                for j in range(0, nk, vstep):
                    js = slice(j, j + vstep)
                    _copy("v", wbf_t[k0][:, js, 0:DVE_W],
                          w8_t[k0][:, js, 0:DVE_W])
                # ACT per k-pair (its per-instruction overhead is high,
                # but whole-group instrs stall the chunk1/2 matmuls)
                astep = 2 if nk >= 2 else 1
                for j in range(0, nk, astep):
                    js = slice(j, j + astep)
                    _copy("a", wbf_t[k0][:, js, DVE_W:O_PER],
                          w8_t[k0][:, js, DVE_W:O_PER])
            wbf_tail = {}
            for i, (o, w) in enumerate(CHUNKS):
                for k in K_TAIL:
                    t = wbfp.tile([128, w], dt.bfloat16, tag=f"wbft_{i}_{k}")
                    _copy(TAIL_ENG[i], t[:], w8_tail[(i, k)][:])
                    wbf_tail[(i, k)] = t

            # ---- matmuls
            psums = [
                psump.tile([16, w], dt.float32, name=f"ps{i}", tag=f"ps{i}")
                for i, (_, w) in enumerate(CHUNKS)
            ]
            for k0, nk in SCHED:
                for j in range(nk):
                    k = k0 + j
                    for i, (o, w) in enumerate(CHUNKS):
                        nc.tensor.matmul(
                            psums[i][:],
                            x_sb[:, k * M:(k + 1) * M],
                            wbf_t[k0][:, j, o:o + w],
                            start=(k == 0),
                            stop=False,
                        )
            # tail, chunk-major so each chunk closes in turn
            for i, (o, w) in enumerate(CHUNKS):
                for k in K_TAIL:
                    nc.tensor.matmul(
                        psums[i][:],
                        x_sb[:, k * M:(k + 1) * M],
                        wbf_tail[(i, k)][:],
                        start=False,
                        stop=(k == K_TAIL[-1]),
                    )

            # ---- epilogue per chunk: PSUM already holds s*(x@W), so one
            # DVE add of the f32 bias closes the chunk; SP DMAs it out.
            for i, (o, w) in enumerate(CHUNKS):
                comb = outp.tile([BATCH, w], dt.float32, tag=f"comb_{i}")
                nc.vector.tensor_add(comb[:], psums[i][:], bias_sb[:, o:o + w])
                nc.sync.dma_start(out[:][:, o:o + w], comb[:])

    nc.compile()
    return nc


def _get_built():
    global _BUILT
    if _BUILT is None:
        _BUILT = _build()
    return _BUILT


def make_in_maps(x, w_q, scale, bias):
    """Host-side shard + layout prep. Returns per-core input dicts."""
    x = np.asarray(x, dtype=np.float32)
    w_q = np.asarray(w_q, dtype=np.int32)
    scale = np.asarray(scale, dtype=np.float32)
    bias = np.asarray(bias, dtype=np.float32)

    # x -> bf16 with the dequant scale folded in, packed so partition p
    # holds, for each k-tile t, the stationary row (t*128 + p): [128, 32*16]
    s_val = scale.reshape(-1)[0]
    xT = np.ascontiguousarray(x.T * s_val).astype(ml_dtypes.bfloat16)
    xt = np.ascontiguousarray(
        xT.reshape(K_TILES, 128, M).transpose(1, 0, 2)
    ).reshape(128, K_TILES * M)

    # codes -> int8 (lossless: w_q in [0,255], shift to [-128,127])
    w8_full = (w_q - 128).astype(np.int8)  # [11008, 4096]

    in_maps = []
    for c in range(N_CORES):
        sl = w8_full[c * O_PER:(c + 1) * O_PER]  # [1376, 4096]
        # [128, 32, 1376]: partition p, (k, f) = W[f, k*128 + p]
        w8c = np.ascontiguousarray(
            sl.T.reshape(K_TILES, 128, O_PER).transpose(1, 0, 2)
        )
        bias_c = np.ascontiguousarray(
            np.broadcast_to(bias[c * O_PER:(c + 1) * O_PER], (BATCH, O_PER))
        )
        in_maps.append(
            {"w8": w8c, "xt": xt, "bias_rep": bias_c}
        )
    return in_maps


def run(inputs, trace=False):
    """Run on the 8 NeuronCores. Returns (full_output, BassKernelResults)."""
    from concourse.bass_utils import run_bass_kernel_spmd

    in_maps = make_in_maps(**inputs)
    nc = _get_built()
    res = run_bass_kernel_spmd(nc, in_maps, list(range(N_CORES)), trace=trace)
    parts = [np.asarray(res.results[c]["out"]) for c in range(N_CORES)]
    full = np.concatenate(parts, axis=1)[:, :OUT_F].astype(np.float32)
    return full, res


def kernel(**inputs) -> np.ndarray:
    full, _ = run(inputs, trace=False)
    return full
